# revision 31
# baseline (speedup 1.0000x reference)
"""BiMamba (bidirectional Mamba2) Trainium2 kernel.

Sharding: 8 NeuronCores = 2 directions x 4 batch sequences; each core runs
the full Mamba2 block (LN -> in_proj -> conv -> chunked SSM scan -> gated
RMSNorm -> out_proj) for one (direction, batch) pair. Host does the
(cheap) sequence flip for the reverse direction and the final
average + LayerNorm combine.

v2 structure (vs v1): conv on DVE (4 shifted MACs) instead of diag
matmuls; hT/X_t/B_t/g^T transposes via XBAR DMA-transpose instead of PE;
per-chunk scan tensors (exp argument, M-matrix) built as whole
[128, H*CH] ops instead of per-head [128,128] ops; inter-chunk C^T S and
state GEMMs batched N=512; D folded into M's diagonal; norm_w folded
into out_proj weights host-side; RMS rstd applied to out_proj output;
PE warmup chain during LN phase; weights/broadcasts prefetched.
"""
import numpy as np
import concourse.bass as bass
import concourse.tile as tile
from concourse import bacc, mybir
from concourse import bass_utils
from concourse.masks import make_identity

F32 = mybir.dt.float32
F16 = mybir.dt.float16
I32 = mybir.dt.int32
AF = mybir.ActivationFunctionType
ALU = mybir.AluOpType
AX = mybir.AxisListType

L = 1024          # seq len
DM = 1024         # d_model
DI = 2048         # d_inner
H = 32            # nheads
PH = 64           # headdim
NS = 128          # d_state
CONV = 2304       # conv channels
EIN = 4384        # in_proj out dim
EPAD = 4480       # padded (35*128)
TC = 8            # time chunks
CH = 128          # chunk length
EPS = 1e-5
NEG = -30000.0
USE_SILU = True   # real HW has silu act table; CoreSim lacks it


def _fast_rsqrt(nc, pool, out_ap, x_ap, magic_bcast, shape, tag):
    """out = 1/sqrt(x) via int bit-hack + 2 Newton iterations (DVE only).
    x_ap must be positive. shape = (128, n). magic_bcast: int32 AP broadcast
    of 0x5f3759df matching shape."""
    n = shape[1]
    sh = pool.tile([128, n], I32, tag=tag + "_sh")
    nc.vector.tensor_scalar(sh[:], x_ap.bitcast(I32), 1, None,
                            op0=ALU.logical_shift_right)
    y = pool.tile([128, n], F32, tag=tag + "_y")
    nc.vector.scalar_tensor_tensor(y[:].bitcast(I32), magic_bcast, 0,
                                   sh[:], op0=ALU.bypass, op1=ALU.subtract)
    xh = pool.tile([128, n], F32, tag=tag + "_xh")
    nc.vector.tensor_scalar_mul(xh[:], x_ap, 0.5)
    t = pool.tile([128, n], F32, tag=tag + "_t")
    for _ in range(2):
        nc.vector.tensor_tensor(t[:], y[:], y[:], op=ALU.mult)
        nc.vector.tensor_tensor(t[:], t[:], xh[:], op=ALU.mult)
        nc.vector.tensor_scalar(t[:], t[:], -1.0, 1.5, op0=ALU.mult, op1=ALU.add)
        nc.vector.tensor_tensor(y[:], y[:], t[:], op=ALU.mult)
    nc.vector.tensor_copy(out=out_ap, in_=y[:])


def _build(nc):
    u_d = nc.dram_tensor("u", [L, DM], F32, kind="ExternalInput").ap()
    w_in_d = nc.dram_tensor("w_in", [DM, EPAD], F16, kind="ExternalInput").ap()
    w_out_d = nc.dram_tensor("w_out", [DI, DM], F16, kind="ExternalInput").ap()
    conv_wt_d = nc.dram_tensor("conv_wt", [128, 18, 4], F32, kind="ExternalInput").ap()
    conv_bt_d = nc.dram_tensor("conv_bt", [128, 18], F32, kind="ExternalInput").ap()
    dt_bias_d = nc.dram_tensor("dt_bias", [32, 1], F32, kind="ExternalInput").ap()
    a_d = nc.dram_tensor("a_neg", [32, 1], F32, kind="ExternalInput").ap()
    d_diag_d = nc.dram_tensor("d_rep", [128, DI], F16, kind="ExternalInput").ap()
    out_d = nc.dram_tensor("out", [L, DM], F32, kind="ExternalOutput").ap()
    with tile.TileContext(nc) as tc:
        _body(nc, tc, u_d, w_in_d, w_out_d, conv_wt_d, conv_bt_d, dt_bias_d,
              a_d, d_diag_d, out_d)
    nc.compile()
    return nc


def _body(nc, tc, u_d, w_in_d, w_out_d, conv_wt_d, conv_bt_d, dt_bias_d,
          a_d, d_diag_d, out_d):
    from contextlib import ExitStack
    ctx = ExitStack()
    with ctx:
        # ---------- constants / small params (whole-kernel lifetime) ----------
        const_p = ctx.enter_context(tc.tile_pool(name="const", bufs=1))
        ident16 = const_p.tile([128, 128], F16)
        make_identity(nc, ident16)
        ident32 = const_p.tile([128, 128], F32)
        make_identity(nc, ident32)
        magic_t = const_p.tile([128, 1], I32)
        nc.gpsimd.memset(magic_t[:], 0x5F3759DF)
        conv_wt = const_p.tile([128, 18, 4], F32)
        nc.sync.dma_start(conv_wt[:], conv_wt_d[:])
        conv_bt = const_p.tile([128, 18], F32)
        nc.sync.dma_start(conv_bt[:], conv_bt_d[:])
        dt_bias = const_p.tile([32, 1], F32)
        nc.sync.dma_start(dt_bias[:], dt_bias_d[:])
        a_neg = const_p.tile([32, 1], F32)
        nc.sync.dma_start(a_neg[:], a_d[:])
        warm_rhs = const_p.tile([128, 512], F16)
        nc.gpsimd.memset(warm_rhs[:], 0.0)

        # ---------- mid-size residents ----------
        res_p = ctx.enter_context(tc.tile_pool(name="res", bufs=1))
        BT_sb = res_p.tile([128, L], F16)         # [n, t]
        CT_sb = res_p.tile([128, L], F16)         # [n, t]
        B_t = res_p.tile([128, TC, NS], F16)      # [tp, tc, n]
        dt_sb = res_p.tile([32, L], F32)          # [h, t]
        dt_T = res_p.tile([128, TC, H], F16)      # [tp, tc, h]
        w_T = res_p.tile([128, TC, H], F16)       # decay-to-chunk-end
        cA_row = res_p.tile([32, TC, CH], F32)    # [h, tc, t]
        cA_colneg = res_p.tile([128, TC, H], F32)  # [tp, tc, h] = -cA
        colmix = res_p.tile([128, TC, H], F32)     # -cA[s] + ln dt[s]
        E0col = res_p.tile([128, TC, H], F16)      # exp(cA[t])
        D_rep = res_p.tile([128, DI], F16)         # D_h replicated per channel
        nc.sync.dma_start(D_rep[:], d_diag_d[:])
        # X layout: [t_lo, ec(c-block), tc, c_lo] so each per-ec DMA-transpose
        # writes a contiguous [128, 8, 128] region (xbar needs contiguous dst)
        X_t = res_p.tile([128, 16, TC, 128], F16)  # 4 MB
        w_out_sb = res_p.tile([128, 16, DM], F16)  # [ep, eo, d]  4 MB

        _uid = nc.next_id()
        sz_dram = nc.dram_tensor(f"sz_spill_{_uid}", [TC, 128, DI], F16).ap()
        ca_dram = nc.dram_tensor(f"ca_bcast_{_uid}", [TC, H * CH], F32).ap()
        daend_dram = nc.dram_tensor(f"daend_bcast_{_uid}", [TC, H], F16).ap()

        with tc.tile_pool(name="hTp", bufs=1) as hTp, \
             tc.tile_pool(name="wzp", bufs=1) as wzp:
            # hT layout: [d_lo, tc, kd, t_lo] — per-chunk transpose writes the
            # contiguous [128, 8, 128] block hT[:, tc]
            hT = hTp.tile([128, TC, 8, 128], F16)  # 2 MB
            # prefetch z weights (4 MB) for phase 3
            wz = wzp.tile([128, 4, 8, 512], F16)
            for eq in range(4):
                nc.gpsimd.dma_start(wz[:, eq], w_in_d[:, eq * 512:(eq + 1) * 512]
                                    .rearrange("(kd p) e -> p kd e", p=128))

            # ---------- phase 0: PE warmup chain (keep HAM at K=8/8) ----------
            with tc.tile_pool(name="warm", bufs=1, space="PSUM") as warmp:
                wps = warmp.tile([128, 512], F32, tag="wps")
                for i in range(40):
                    nc.tensor.matmul(wps[:], lhsT=ident16[:], rhs=warm_rhs[:],
                                     start=(i == 0), stop=(i == 39))

            # ---------- phase 1: LN(u) -> h (f16), DMA-transpose -> hT ----------
            with tc.tile_pool(name="ph1u", bufs=8) as p1u, \
                 tc.tile_pool(name="ph1", bufs=3) as p1, \
                 tc.tile_pool(name="ph1s", bufs=3) as p1s:
                u_ts = []
                for t_c in range(TC):
                    u_t = p1u.tile([128, DM], F32, tag="u", name=f"u{t_c}")
                    nc.gpsimd.dma_start(u_t[:], u_d[t_c * 128:(t_c + 1) * 128, :])
                    u_ts.append(u_t)
                for t_c in range(TC):
                    u_t = u_ts[t_c]
                    ssum = p1s.tile([128, 1], F32, tag="ssum")
                    nc.vector.tensor_reduce(ssum[:], u_t[:], axis=AX.X, op=ALU.add)
                    sq = p1.tile([128, DM], F32, tag="sq")
                    ssq = p1s.tile([128, 1], F32, tag="ssq")
                    nc.scalar.activation(sq[:], u_t[:], AF.Square, accum_out=ssq[:])
                    nmean = p1s.tile([128, 1], F32, tag="nmean")
                    nc.vector.tensor_scalar_mul(nmean[:], ssum[:], -1.0 / DM)
                    var = p1s.tile([128, 1], F32, tag="var")
                    nc.vector.tensor_tensor(var[:], nmean[:], nmean[:], op=ALU.mult)
                    nc.vector.scalar_tensor_tensor(var[:], ssq[:], 1.0 / DM, var[:],
                                                   op0=ALU.mult, op1=ALU.subtract)
                    nc.vector.tensor_scalar_add(var[:], var[:], EPS)
                    rstd = p1s.tile([128, 1], F32, tag="rstd")
                    _fast_rsqrt(nc, p1s, rstd[:], var[:], magic_t[:], (128, 1), "ln")
                    bias2 = p1s.tile([128, 1], F32, tag="bias2")
                    nc.vector.tensor_tensor(bias2[:], nmean[:], rstd[:], op=ALU.mult)
                    h_t = p1.tile([128, DM], F16, tag="h")
                    nc.vector.tensor_scalar(h_t[:], u_t[:], rstd[:], bias2[:],
                                            op0=ALU.mult, op1=ALU.add)
                    nc.sync.dma_start(hT[:, t_c], h_t[:], transpose=True)

            # ---------- phase 2: in_proj xBC/dt + DVE conv + DMA transposes ----
            with tc.tile_pool(name="p2w", bufs=6) as wp, \
                 tc.tile_pool(name="p2", bufs=3) as p2, \
                 tc.tile_pool(name="p2ps", bufs=2, space="PSUM") as pps:
                for ec in range(19):
                    e0 = DI + ec * 128
                    m = 128 if ec < 18 else 32
                    ps = pps.tile([128, 2, 512], F32, tag="px")
                    wt = wp.tile([128, 8, 128], F16, tag="w")
                    nc.gpsimd.dma_start(wt[:], w_in_d[:, e0:e0 + 128]
                                        .rearrange("(kd p) e -> p kd e", p=128))
                    for th in range(2):
                        for kd in range(8):
                            nc.tensor.matmul(ps[:m, th], lhsT=wt[:, kd, :m],
                                             rhs=hT[:, th * 4:(th + 1) * 4, kd, :],
                                             start=(kd == 0), stop=(kd == 7))
                    if ec == 18:
                        # softplus(x + dt_bias) = ln(1 + exp(x + dt_bias))
                        nc.scalar.activation(dt_sb[:], ps[:32].rearrange("p a b -> p (a b)"),
                                             AF.Exp, bias=dt_bias[:])
                        nc.scalar.activation(dt_sb[:], dt_sb[:], AF.Ln, bias=1.0)
                        continue
                    xr = p2.tile([128, 3 + L], F16, tag="xraw")
                    nc.gpsimd.memset(xr[:, 0:3], 0.0)
                    nc.scalar.activation(xr[:, 3:3 + L], ps[:].rearrange("p a b -> p (a b)"),
                                         AF.Copy)
                    # depthwise causal conv: 4 shifted MACs split DVE/Pool
                    xca = p2.tile([128, L], F16, tag="xconva")
                    nc.vector.tensor_scalar_mul(xca[:], xr[:, 0:L], conv_wt[:, ec, 0:1])
                    nc.vector.scalar_tensor_tensor(xca[:], xr[:, 1:1 + L],
                                                   conv_wt[:, ec, 1:2], xca[:],
                                                   op0=ALU.mult, op1=ALU.add)
                    xcb = p2.tile([128, L], F16, tag="xconvb")
                    nc.gpsimd.tensor_scalar(xcb[:], xr[:, 2:2 + L], conv_wt[:, ec, 2:3],
                                            None, op0=ALU.mult)
                    xcc = p2.tile([128, L], F16, tag="xconvc")
                    nc.gpsimd.tensor_scalar(xcc[:], xr[:, 3:3 + L], conv_wt[:, ec, 3:4],
                                            None, op0=ALU.mult)
                    nc.gpsimd.tensor_tensor(xcb[:], xcb[:], xcc[:], op=ALU.add)
                    xc = p2.tile([128, L], F16, tag="xconv")
                    nc.vector.tensor_tensor(xc[:], xca[:], xcb[:], op=ALU.add)
                    def _silu_conv(dst):
                        if USE_SILU:
                            nc.scalar.activation(dst, xc[:], AF.Silu,
                                                 bias=conv_bt[:, ec:ec + 1])
                        else:
                            sg = p2.tile([128, L], F16, tag="sg")
                            nc.scalar.activation(sg[:], xc[:], AF.Sigmoid,
                                                 bias=conv_bt[:, ec:ec + 1])
                            nc.vector.scalar_tensor_tensor(dst, xc[:],
                                                           conv_bt[:, ec:ec + 1],
                                                           sg[:], op0=ALU.add, op1=ALU.mult)
                    if ec <= 15:
                        xa = p2.tile([128, L], F16, tag="xact")
                        _silu_conv(xa[:])
                        nc.sync.dma_start(X_t[:, ec], xa[:], transpose=True)
                    elif ec == 16:
                        _silu_conv(BT_sb[:])
                        nc.sync.dma_start(B_t[:], BT_sb[:], transpose=True)
                    else:
                        _silu_conv(CT_sb[:])

            # ---------- phase 3: z GEMM -> silu_z -> spill ----------
            with tc.tile_pool(name="p3b", bufs=4) as p3b, \
                 tc.tile_pool(name="p3ps", bufs=2, space="PSUM") as pz:
                # prefetch out_proj weights during phase 3
                nc.gpsimd.dma_start(w_out_sb[:], w_out_d.rearrange("(eo p) d -> p eo d", p=128))
                for eq in range(4):
                    for tg in range(4):
                        pss = pz.tile([128, 2, 512], F32, tag="pz")
                        for j in range(2):
                            t_c = tg * 2 + j
                            for kd in range(8):
                                nc.tensor.matmul(pss[:, j],
                                                 lhsT=hT[:, t_c, kd, :],
                                                 rhs=wz[:, eq, kd],
                                                 start=(kd == 0), stop=(kd == 7))
                        for j in range(2):
                            t_c = tg * 2 + j
                            zb = p3b.tile([128, 512], F16, tag="zb")
                            if USE_SILU:
                                nc.scalar.activation(zb[:], pss[:, j], AF.Silu)
                            else:
                                zsg = p3b.tile([128, 512], F16, tag="zsg")
                                nc.scalar.activation(zsg[:], pss[:, j], AF.Sigmoid)
                                nc.vector.tensor_tensor(zb[:], pss[:, j], zsg[:], op=ALU.mult)
                            nc.gpsimd.dma_start(sz_dram[t_c, :, eq * 512:(eq + 1) * 512], zb[:])
        # hT + wz pools closed here

        # ---------- dt/cA machinery ----------
        with tc.tile_pool(name="pdt", bufs=2) as pdt, \
             tc.tile_pool(name="pdtps", bufs=4, space="PSUM") as pdtps:
            dtA = pdt.tile([32, L], F32, tag="dtA")
            nc.vector.tensor_scalar_mul(dtA[:], dt_sb[:], a_neg[:])
            for t_c in range(TC):
                sl = slice(t_c * 128, (t_c + 1) * 128)
                nc.vector.tensor_tensor_scan(cA_row[:, t_c, :], dtA[:, sl], dtA[:, sl],
                                             initial=0.0, op0=ALU.add, op1=ALU.bypass)
                pdts = pdtps.tile([128, 32], F32, tag="pq")
                nc.tensor.transpose(pdts[:], dt_sb[:, sl], ident32[:32, :32])
                nc.any.tensor_copy(out=dt_T[:, t_c, :], in_=pdts[:])
                wr = pdt.tile([32, CH], F32, tag="wr")
                nc.scalar.activation(wr[:], cA_row[:, t_c, :], AF.Exp, scale=-1.0,
                                     bias=cA_row[:, t_c, 127:128])
                pdts2 = pdtps.tile([128, 32], F32, tag="pq")
                nc.tensor.transpose(pdts2[:], wr[:], ident32[:32, :32])
                nc.any.tensor_copy(out=w_T[:, t_c, :], in_=pdts2[:])
                pdts3 = pdtps.tile([128, 32], F32, tag="pq")
                nc.tensor.transpose(pdts3[:], cA_row[:, t_c, :], ident32[:32, :32])
                nc.vector.tensor_scalar_mul(cA_colneg[:, t_c, :], pdts3[:], -1.0)
                nc.sync.dma_start(ca_dram[t_c], cA_row[:, t_c, :])
                daend_2d = pdt.tile([32, 1], F16, tag="daend_2d")
                nc.scalar.activation(daend_2d[:], cA_row[:, t_c, 127:128], AF.Exp)
                nc.sync.dma_start(daend_dram[t_c], daend_2d[:])
            # ln(dt) in [tp, tc, h]; colmix = -cA[s] + ln dt[s]; E0 = exp(cA[t])
            lndt = pdt.tile([128, TC, H], F32, tag="lndt")
            nc.scalar.activation(lndt[:].rearrange("p a b -> p (a b)"),
                                 dt_T[:].rearrange("p a b -> p (a b)"), AF.Ln)
            nc.vector.tensor_tensor(colmix[:], lndt[:], cA_colneg[:], op=ALU.add)
            nc.scalar.activation(E0col[:].rearrange("p a b -> p (a b)"),
                                 cA_colneg[:].rearrange("p a b -> p (a b)"),
                                 AF.Exp, scale=-1.0)

        # ---------- phase 4: scan + gating + rmsnorm + out_proj ----------
        with tc.tile_pool(name="p4ca", bufs=1) as p4ca, \
             tc.tile_pool(name="p4rep", bufs=2) as p4rep, \
             tc.tile_pool(name="p4sz", bufs=2) as p4sz, \
             tc.tile_pool(name="p4big", bufs=1) as p4big, \
             tc.tile_pool(name="p4mt", bufs=2) as p4mt, \
             tc.tile_pool(name="p4s", bufs=2) as p4s, \
             tc.tile_pool(name="p4x", bufs=2) as p4x, \
             tc.tile_pool(name="p4g", bufs=2) as p4g, \
             tc.tile_pool(name="p4y", bufs=2) as p4y, \
             tc.tile_pool(name="psgt", bufs=1, space="PSUM") as psgt, \
             tc.tile_pool(name="psy", bufs=1, space="PSUM") as psy, \
             tc.tile_pool(name="psst", bufs=1, space="PSUM") as psst, \
             tc.tile_pool(name="pso", bufs=1, space="PSUM") as pso:
            def fetch(i, tag_sfx=""):
                car = p4ca.tile([128, H, CH], F32, tag="carep")
                nc.sync.dma_start(car[:].rearrange("p a b -> p (a b)"),
                                  ca_dram[i:i + 1, :].partition_broadcast(128)[:, 0, :])
                dae = p4rep.tile([128, H], F16, tag="daendrep")
                nc.sync.dma_start(dae[:],
                                  daend_dram[i:i + 1, :].partition_broadcast(128)[:, 0, :])
                szb = p4sz.tile([128, DI], F16, tag="szsb")
                nc.scalar.dma_start(szb[:], sz_dram[i])
                return car, dae, szb

            nxt = fetch(0)
            S_prev = None
            for t_c in range(TC):
                tsl = slice(t_c * 128, (t_c + 1) * 128)
                ca_rep, daend_rep, sz_sb = nxt
                if t_c + 1 < TC:
                    nxt = fetch(t_c + 1)
                # add colmix -> f16 (split DVE/Pool), then causal mask (Pool)
                ca16 = p4big.tile([128, H, CH], F16, tag="ca16")
                nc.vector.tensor_tensor(
                    ca16[:, 0:16, :], ca_rep[:, 0:16, :],
                    colmix[:, t_c, 0:16, None].to_broadcast((128, 16, CH)),
                    op=ALU.add)
                nc.gpsimd.tensor_tensor(
                    ca16[:, 16:32, :], ca_rep[:, 16:32, :],
                    colmix[:, t_c, 16:32, None].to_broadcast((128, 16, CH)),
                    op=ALU.add)
                nc.gpsimd.affine_select(out=ca16[:], in_=ca16[:],
                                        pattern=[[0, H], [1, CH]],
                                        compare_op=ALU.is_ge, fill=-1000.0, base=0,
                                        channel_multiplier=-1)
                Ep = p4big.tile([128, H, CH], F16, tag="Ep")
                nc.scalar.activation(Ep[:].rearrange("p a b -> p (a b)"),
                                     ca16[:].rearrange("p a b -> p (a b)"), AF.Exp)
                # Gt (shared across heads)
                gt_ps = psgt.tile([128, CH], F32, tag="gt")
                nc.tensor.matmul(gt_ps[:], lhsT=BT_sb[:, tsl], rhs=CT_sb[:, tsl],
                                 start=True, stop=True)
                gt_sb = p4x.tile([128, CH], F16, tag="gts")
                nc.scalar.activation(gt_sb[:], gt_ps[:], AF.Copy)
                # M = E' * gt  (split DVE/Pool; D handled in gating)
                Mt = p4mt.tile([128, H, CH], F16, tag="Mt")
                nc.vector.tensor_tensor(Mt[:, 0:16, :], Ep[:, 0:16, :],
                                        gt_sb[:, None, :].to_broadcast((128, 16, CH)),
                                        op=ALU.mult)
                nc.gpsimd.tensor_tensor(Mt[:, 16:32, :], Ep[:, 16:32, :],
                                        gt_sb[:, None, :].to_broadcast((128, 16, CH)),
                                        op=ALU.mult)
                # x * dt * decay-to-end
                dtw = p4x.tile([128, H], F16, tag="dtw")
                nc.vector.tensor_tensor(dtw[:], dt_T[:, t_c, :], w_T[:, t_c, :],
                                        op=ALU.mult)
                xch = p4x.tile([128, H, PH], F16, tag="xchk")
                nc.vector.tensor_tensor(
                    xch[:].rearrange("p (a b) q -> p a b q", a=16),
                    X_t[:, :, t_c, :].rearrange("p a (b q) -> p a b q", q=PH),
                    dtw[:].rearrange("p (a b) -> p a b", a=16)[:, :, :, None]
                    .to_broadcast((128, 16, 2, PH)),
                    op=ALU.mult)
                S_new = p4s.tile([128, H, PH], F16, tag="S")
                if t_c > 0:
                    nc.gpsimd.tensor_tensor(S_new[:], S_prev[:],
                                            daend_rep[:, :, None].to_broadcast((128, H, PH)),
                                            op=ALU.mult)
                g16 = p4g.tile([128, DI], F16, tag="g16")
                for q in range(2):
                    hsl = slice(q * 16, (q + 1) * 16)
                    esl = slice(q * 1024, (q + 1) * 1024)
                    # state GEMM for this half
                    st = psst.tile([128, 2, 512], F32, tag="st")
                    for j in range(2):
                        nc.tensor.matmul(st[:, j, :], lhsT=B_t[:, t_c, :],
                                         rhs=xch[:].rearrange("p a b -> p (a b)")[:, q * 1024 + j * 512: q * 1024 + (j + 1) * 512],
                                         start=True, stop=True)
                    if t_c == 0:
                        nc.vector.tensor_copy(out=S_new[:, hsl, :],
                                              in_=st[:].rearrange("p a (h q) -> p (a h) q", q=PH))
                    else:
                        nc.vector.tensor_tensor(S_new[:, hsl, :], S_new[:, hsl, :],
                                                st[:].rearrange("p a (h q) -> p (a h) q", q=PH),
                                                op=ALU.add)
                    # intra-chunk: y1 = M^T x per head
                    y1_ps = psy.tile([128, 16, PH], F32, tag="y1")
                    for hh in range(16):
                        h = q * 16 + hh
                        nc.tensor.matmul(y1_ps[:, hh, :], lhsT=Mt[:, h, :],
                                         rhs=X_t[:, h // 2, t_c, (h % 2) * PH:(h % 2 + 1) * PH],
                                         start=True, stop=True)
                    xD = p4y.tile([128, 16, PH], F16, tag="xD")
                    nc.gpsimd.tensor_tensor(
                        xD[:].rearrange("p a b -> p (a b)").rearrange("p (a b) -> p a b", a=8),
                        X_t[:, q * 8:(q + 1) * 8, t_c, :],
                        D_rep[:, esl].rearrange("p (a b) -> p a b", a=8), op=ALU.mult)
                    if t_c > 0:
                        # inter-chunk: y2 = C^T S_prev, batched over heads
                        y2_ps = psy.tile([128, 2, 512], F32, tag="y2")
                        for j in range(2):
                            nc.tensor.matmul(y2_ps[:, j, :], lhsT=CT_sb[:, tsl],
                                             rhs=S_prev[:].rearrange("p a b -> p (a b)")[:, q * 1024 + j * 512: q * 1024 + (j + 1) * 512],
                                             start=True, stop=True)
                        y2c = p4y.tile([128, 16, PH], F16, tag="y2c")
                        nc.scalar.activation(y2c[:].rearrange("p a b -> p (a b)"),
                                             y2_ps[:].rearrange("p a b -> p (a b)"),
                                             AF.Copy)
                        t1 = p4y.tile([128, 16, PH], F16, tag="t1")
                        nc.vector.tensor_tensor(
                            t1[:], y2c[:],
                            E0col[:, t_c, hsl, None].to_broadcast((128, 16, PH)),
                            op=ALU.mult)
                        nc.vector.tensor_tensor(t1[:], t1[:],
                                                y1_ps[:].rearrange("p a (h q) -> p (a h) q", q=PH),
                                                op=ALU.add)
                        nc.vector.tensor_tensor(t1[:], t1[:], xD[:], op=ALU.add)
                        nc.vector.tensor_tensor(g16[:, esl].rearrange("p (a b) -> p a b", a=16),
                                                t1[:], sz_sb[:, esl].rearrange("p (a b) -> p a b", a=16),
                                                op=ALU.mult)
                    else:
                        t1 = p4y.tile([128, 16, PH], F16, tag="t1")
                        nc.vector.tensor_tensor(t1[:],
                                                y1_ps[:].rearrange("p a (h q) -> p (a h) q", q=PH),
                                                xD[:], op=ALU.add)
                        nc.vector.tensor_tensor(g16[:, esl].rearrange("p (a b) -> p a b", a=16),
                                                t1[:], sz_sb[:, esl].rearrange("p (a b) -> p a b", a=16),
                                                op=ALU.mult)
                S_prev = S_new
                # --- rmsnorm stats (rstd applied at out_proj output) ---
                gsq = p4g.tile([128, 1024], F32, tag="gsq")
                sq1 = p4g.tile([128, 1], F32, tag="sq1")
                sq2 = p4g.tile([128, 1], F32, tag="sq2")
                nc.scalar.activation(gsq[:], g16[:, 0:1024], AF.Square, accum_out=sq1[:])
                nc.scalar.activation(gsq[:], g16[:, 1024:2048], AF.Square, accum_out=sq2[:])
                nc.vector.tensor_tensor(sq1[:], sq1[:], sq2[:], op=ALU.add)
                msq = p4g.tile([128, 1], F32, tag="msq")
                nc.vector.tensor_scalar(msq[:], sq1[:], 1.0 / DI, EPS,
                                        op0=ALU.mult, op1=ALU.add)
                rstd = p4g.tile([128, 1], F32, tag="rstd")
                _fast_rsqrt(nc, p4g, rstd[:], msq[:], magic_t[:], (128, 1), "rms")
                # --- transpose g16 -> [e, t] via DMA xbar ---
                gT = p4y.tile([128, 16, CH], F16, tag="gT")
                nc.sync.dma_start(gT[:], g16[:], transpose=True)
                # --- out_proj (+ deferred rstd scale) ---
                for dh in range(2):
                    po = pso.tile([128, 512], F32, tag="po")
                    for eo in range(16):
                        nc.tensor.matmul(po[:], lhsT=gT[:, eo, :],
                                         rhs=w_out_sb[:, eo, dh * 512:(dh + 1) * 512],
                                         start=(eo == 0), stop=(eo == 15))
                    ob = p4y.tile([128, 512], F32, tag="ob")
                    nc.scalar.activation(ob[:], po[:], AF.Copy, scale=rstd[:])
                    nc.gpsimd.dma_start(out_d[tsl, dh * 512:(dh + 1) * 512], ob[:])


_NC_CACHE = {}

N_CORES = 8
BSZ = 4


def _get_nc():
    if "nc" not in _NC_CACHE:
        nc = bacc.Bacc("TRN2", target_bir_lowering=False, debug=False,
                       num_devices=N_CORES)
        _NC_CACHE["nc"] = _build(nc)
    return _NC_CACHE["nc"]


def _get_runner():
    """Build the jitted SPMD callable once so repeat kernel() calls skip
    retrace + NEFF recompile (run_bass_via_pjrt builds a fresh closure per
    call, defeating the jit cache)."""
    if "runner" not in _NC_CACHE:
        _NC_CACHE["runner"] = _make_runner(_get_nc())
    return _NC_CACHE["runner"]


def _make_runner(nc):
    import jax
    from jax.sharding import Mesh, PartitionSpec
    from jax.experimental.shard_map import shard_map
    from concourse import bass2jax, mybir as _mb

    bass2jax.install_neuronx_cc_hook()
    partition_name = nc.partition_id_tensor.name if nc.partition_id_tensor else None
    in_names, out_names, out_avals, zero_outs = [], [], [], []
    for alloc in nc.m.functions[0].allocations:
        if not isinstance(alloc, _mb.MemoryLocationSet):
            continue
        name = alloc.memorylocations[0].name
        if alloc.kind == "ExternalInput":
            if name != partition_name:
                in_names.append(name)
        elif alloc.kind == "ExternalOutput":
            shape = tuple(alloc.tensor_shape)
            dtype = _mb.dt.np(alloc.dtype)
            out_names.append(name)
            out_avals.append(jax.core.ShapedArray(shape, dtype))
            zero_outs.append(np.zeros(shape, dtype))
    n_params = len(in_names)
    n_outs = len(out_avals)
    all_in_names = list(in_names) + list(out_names)
    if partition_name is not None:
        all_in_names.append(partition_name)
    donate = tuple(range(n_params, n_params + n_outs))

    def _bodyfn(*args):
        operands = list(args)
        if partition_name is not None:
            operands.append(bass2jax.partition_id_tensor())
        outs = bass2jax._bass_exec_p.bind(
            *operands,
            out_avals=tuple(out_avals),
            in_names=tuple(all_in_names),
            out_names=tuple(out_names),
            lowering_input_output_aliases=(),
            sim_require_finite=True,
            sim_require_nnan=True,
            nc=nc,
        )
        return tuple(outs)

    devices = jax.devices()[:N_CORES]
    mesh = Mesh(np.asarray(devices), ("core",))
    in_specs = (PartitionSpec("core"),) * (n_params + n_outs)
    out_specs = (PartitionSpec("core"),) * n_outs
    sharded = jax.jit(
        shard_map(_bodyfn, mesh=mesh, in_specs=in_specs, out_specs=out_specs,
                  check_rep=False),
        donate_argnums=donate, keep_unused=True)

    def run(in_maps):
        per_core = [[np.asarray(m[name]) for name in in_names] for m in in_maps]
        concat_in = [np.concatenate([per_core[c][i] for c in range(N_CORES)], axis=0)
                     for i in range(n_params)]
        concat_zeros = [np.zeros((N_CORES * z.shape[0], *z.shape[1:]), z.dtype)
                        for z in zero_outs]
        out_arrs = sharded(*concat_in, *concat_zeros)
        return [{name: np.asarray(out_arrs[i]).reshape(N_CORES, *out_avals[i].shape)[c]
                 for i, name in enumerate(out_names)}
                for c in range(N_CORES)]

    def make_device_exec(in_maps):
        """For timing: stage inputs on-device once; returns f() that runs one
        execution with on-device zero outputs and blocks until done."""
        from jax.sharding import NamedSharding
        per_core = [[np.asarray(m[name]) for name in in_names] for m in in_maps]
        concat_in = [np.concatenate([per_core[c][i] for c in range(N_CORES)], axis=0)
                     for i in range(n_params)]
        shard = NamedSharding(mesh, PartitionSpec("core"))
        dev_in = [jax.device_put(a, shard) for a in concat_in]
        zero_shapes = [(N_CORES * z.shape[0], *z.shape[1:]) for z in zero_outs]
        zdtypes = [z.dtype for z in zero_outs]
        import jax.numpy as jnp
        mk_zeros = jax.jit(
            lambda: tuple(jnp.zeros(s, d) for s, d in zip(zero_shapes, zdtypes)),
            out_shardings=tuple(shard for _ in zero_shapes))

        def exec_once():
            zs = mk_zeros()
            jax.block_until_ready(zs)
            import time as _t
            t0 = _t.perf_counter()
            outs = sharded(*dev_in, *zs)
            jax.block_until_ready(outs)
            return _t.perf_counter() - t0
        return exec_once

    run.make_device_exec = make_device_exec
    return run


def _smart_flip(X, lengths):
    B, Ln, _ = X.shape
    r = np.arange(Ln)[None, :]
    pos = np.where(r < lengths[:, None], lengths[:, None] - 1 - r, r)
    return np.take_along_axis(X, pos[:, :, None], axis=1)


def _dir_params(in_proj_w, out_proj_w, conv_w, conv_b, dt_bias, A_log, D, norm_w):
    w_in = np.zeros((DM, EPAD), np.float16)
    w_in[:, :EIN] = in_proj_w.T.astype(np.float16)
    # out_proj with norm_w folded in: w_out[e, d] = norm_w[e] * out_proj_w[d, e]
    w_out = np.ascontiguousarray(out_proj_w.T).astype(np.float64)
    w_out = (w_out * norm_w.astype(np.float64)[:, None]).astype(np.float16)
    d_rep = np.broadcast_to(
        np.repeat(D.astype(np.float16), PH)[None, :], (128, DI)).copy()
    return {
        "w_in": w_in,
        "w_out": w_out,
        "conv_wt": np.ascontiguousarray(
            conv_w.reshape(18, 128, 4).transpose(1, 0, 2)).astype(np.float32),
        "conv_bt": np.ascontiguousarray(conv_b.reshape(18, 128).T.astype(np.float32)),
        "dt_bias": dt_bias.reshape(32, 1).astype(np.float32),
        "a_neg": (-np.exp(A_log.astype(np.float64))).astype(np.float32).reshape(32, 1),
        "d_rep": d_rep,
    }


def kernel(hidden_states, src_key_padding_mask, in_proj_w, out_proj_w,
           conv_w_f, conv_b_f, dt_bias_f, A_log_f, D_f, norm_w_f,
           conv_w_r, conv_b_r, dt_bias_r, A_log_r, D_r, norm_w_r):
    hidden_states = np.asarray(hidden_states, np.float32)
    mask = np.asarray(src_key_padding_mask)
    lengths = (~mask).sum(axis=1)
    rev = _smart_flip(hidden_states, lengths)

    pf = _dir_params(np.asarray(in_proj_w), np.asarray(out_proj_w),
                     np.asarray(conv_w_f), np.asarray(conv_b_f),
                     np.asarray(dt_bias_f), np.asarray(A_log_f),
                     np.asarray(D_f), np.asarray(norm_w_f))
    pr = _dir_params(np.asarray(in_proj_w), np.asarray(out_proj_w),
                     np.asarray(conv_w_r), np.asarray(conv_b_r),
                     np.asarray(dt_bias_r), np.asarray(A_log_r),
                     np.asarray(D_r), np.asarray(norm_w_r))

    run = _get_runner()
    in_maps = []
    for core in range(N_CORES):
        d, b = divmod(core, BSZ)
        u = hidden_states[b] if d == 0 else rev[b]
        m = dict(pf if d == 0 else pr)
        m["u"] = np.ascontiguousarray(u)
        in_maps.append(m)
    results = run(in_maps)
    out_f = np.stack([results[b]["out"] for b in range(BSZ)])
    out_r = np.stack([results[BSZ + b]["out"] for b in range(BSZ)])
    out_r = _smart_flip(out_r, lengths)
    out = (out_f.astype(np.float64) + out_r.astype(np.float64)) / 2.0
    mu = out.mean(-1, keepdims=True)
    v = out.var(-1, keepdims=True)
    out = (out - mu) / np.sqrt(v + EPS)
    return out.astype(np.float32)


# revision 43
# speedup vs baseline: 1.1352x; 1.1352x over previous
"""BiMamba (bidirectional Mamba2) Trainium2 kernel.

Sharding: 8 NeuronCores = 2 directions x 4 batch sequences; each core runs
the full Mamba2 block (LN -> in_proj -> conv -> chunked SSM scan -> gated
RMSNorm -> out_proj) for one (direction, batch) pair. Host does the
(cheap) sequence flip for the reverse direction and the final
average + LayerNorm combine.

v3 structure (vs v1 baseline): conv via 4 shifted MACs on DVE instead of
diag matmuls on PE; per-chunk scan tensors (exp argument, M-matrix,
gating) built as whole [128, H*CH] ops instead of per-head [128,128]
ops (exp count: 8 instead of 256); inter-chunk C^T S and state GEMMs
batched N=512 with shared stationary operand; D applied in gating via a
replicated D tile; norm_w folded into out_proj weights host-side; RMS
rstd deferred to a per-partition scale on the out_proj output; causal
mask via one affine_select per chunk on gpsimd; PE warmup chain keeps
HAM at full clock through the LN phase; u/weights/broadcast loads
prefetched on separate DMA queues (gpsimd=bulk weights, scalar=wt/sz,
sync=small + stores).  Transposes stay on the PE (XBAR DMA-transpose
measured ~25 GB/s/ring — too slow; Pool-engine elementwise ops measured
2-15x slower than DVE — only affine_select/memset/DMA live there).
"""
import numpy as np
import concourse.bass as bass
import concourse.tile as tile
from concourse import bacc, mybir
from concourse import bass_utils
from concourse.masks import make_identity

F32 = mybir.dt.float32
F16 = mybir.dt.float16
I32 = mybir.dt.int32
AF = mybir.ActivationFunctionType
ALU = mybir.AluOpType
AX = mybir.AxisListType

L = 1024          # seq len
DM = 1024         # d_model
DI = 2048         # d_inner
H = 32            # nheads
PH = 64           # headdim
NS = 128          # d_state
CONV = 2304       # conv channels
EIN = 4384        # in_proj out dim
EPAD = 4480       # padded (35*128)
TC = 8            # time chunks
CH = 128          # chunk length
EPS = 1e-5
NEG = -30000.0
USE_SILU = True   # real HW has silu act table; CoreSim lacks it


def _fast_rsqrt(nc, pool, out_ap, x_ap, magic_bcast, shape, tag):
    """out = 1/sqrt(x) via int bit-hack + 2 Newton iterations (DVE only).
    x_ap must be positive. shape = (128, n). magic_bcast: int32 AP broadcast
    of 0x5f3759df matching shape."""
    n = shape[1]
    sh = pool.tile([128, n], I32, tag=tag + "_sh")
    nc.vector.tensor_scalar(sh[:], x_ap.bitcast(I32), 1, None,
                            op0=ALU.logical_shift_right)
    y = pool.tile([128, n], F32, tag=tag + "_y")
    nc.vector.scalar_tensor_tensor(y[:].bitcast(I32), magic_bcast, 0,
                                   sh[:], op0=ALU.bypass, op1=ALU.subtract)
    xh = pool.tile([128, n], F32, tag=tag + "_xh")
    nc.vector.tensor_scalar_mul(xh[:], x_ap, 0.5)
    t = pool.tile([128, n], F32, tag=tag + "_t")
    for _ in range(2):
        nc.vector.tensor_tensor(t[:], y[:], y[:], op=ALU.mult)
        nc.vector.tensor_tensor(t[:], t[:], xh[:], op=ALU.mult)
        nc.vector.tensor_scalar(t[:], t[:], -1.0, 1.5, op0=ALU.mult, op1=ALU.add)
        nc.vector.tensor_tensor(y[:], y[:], t[:], op=ALU.mult)
    nc.vector.tensor_copy(out=out_ap, in_=y[:])


def _build(nc):
    u_d = nc.dram_tensor("u", [L, DM], F32, kind="ExternalInput").ap()
    w_in_d = nc.dram_tensor("w_in", [DM, EPAD], F16, kind="ExternalInput").ap()
    w_out_d = nc.dram_tensor("w_out", [DI, DM], F16, kind="ExternalInput").ap()
    conv_wt_d = nc.dram_tensor("conv_wt", [128, 18, 4], F32, kind="ExternalInput").ap()
    conv_bt_d = nc.dram_tensor("conv_bt", [128, 18], F32, kind="ExternalInput").ap()
    dt_bias_d = nc.dram_tensor("dt_bias", [32, 1], F32, kind="ExternalInput").ap()
    a_d = nc.dram_tensor("a_neg", [32, 1], F32, kind="ExternalInput").ap()
    d_diag_d = nc.dram_tensor("d_rep", [128, DI], F16, kind="ExternalInput").ap()
    out_d = nc.dram_tensor("out", [L, DM], F16, kind="ExternalOutput").ap()
    with tile.TileContext(nc) as tc:
        _body(nc, tc, u_d, w_in_d, w_out_d, conv_wt_d, conv_bt_d, dt_bias_d,
              a_d, d_diag_d, out_d)
    nc.compile()
    return nc


def _body(nc, tc, u_d, w_in_d, w_out_d, conv_wt_d, conv_bt_d, dt_bias_d,
          a_d, d_diag_d, out_d):
    from contextlib import ExitStack
    ctx = ExitStack()
    with ctx:
        # ---------- constants / small params (whole-kernel lifetime) ----------
        const_p = ctx.enter_context(tc.tile_pool(name="const", bufs=1))
        ident16 = const_p.tile([128, 128], F16)
        make_identity(nc, ident16)
        ident32 = const_p.tile([128, 128], F32)
        make_identity(nc, ident32)
        magic_t = const_p.tile([128, 1], I32)
        nc.gpsimd.memset(magic_t[:], 0x5F3759DF)
        conv_wt = const_p.tile([128, 18, 4], F32)
        nc.sync.dma_start(conv_wt[:], conv_wt_d[:])
        conv_bt = const_p.tile([128, 18], F32)
        nc.sync.dma_start(conv_bt[:], conv_bt_d[:])
        dt_bias = const_p.tile([32, 1], F32)
        nc.sync.dma_start(dt_bias[:], dt_bias_d[:])
        a_neg = const_p.tile([32, 1], F32)
        nc.sync.dma_start(a_neg[:], a_d[:])
        warm_rhs = const_p.tile([128, 512], F16)
        nc.gpsimd.memset(warm_rhs[:], 0.0)
        ones_ph = const_p.tile([32, PH], F16)
        nc.gpsimd.memset(ones_ph[:], 1.0)

        # ---------- mid-size residents ----------
        res_p = ctx.enter_context(tc.tile_pool(name="res", bufs=1))
        BT_sb = res_p.tile([128, L], F16)         # [n, t]
        CT_sb = res_p.tile([128, L], F16)         # [n, t]
        B_t = res_p.tile([128, TC, NS], F16)      # [tp, tc, n]
        dt_sb = res_p.tile([32, L], F32)          # [h, t]
        dt_T = res_p.tile([128, TC, H], F16)      # [tp, tc, h]
        w_T = res_p.tile([128, TC, H], F16)       # decay-to-chunk-end
        cA_row = res_p.tile([32, TC, CH], F32)    # [h, tc, t]
        cA_colneg = res_p.tile([128, TC, H], F32)  # [tp, tc, h] = -cA
        colmix = res_p.tile([128, TC, H], F32)     # -cA[s] + ln dt[s]
        E0col = res_p.tile([128, TC, H], F16)      # exp(cA[t])
        D_rep = res_p.tile([128, DI], F16)         # D_h replicated per channel
        nc.scalar.dma_start(D_rep[:], d_diag_d[:])
        # X layout: [t_lo, ec(c-block), tc, c_lo] so each per-ec DMA-transpose
        # writes a contiguous [128, 8, 128] region (xbar needs contiguous dst)
        X_t = res_p.tile([128, 16, TC, 128], F16)  # 4 MB
        w_out_sb = res_p.tile([128, 16, DM], F16)  # [ep, eo, d]  4 MB

        _uid = nc.next_id()
        sz_dram = nc.dram_tensor(f"sz_spill_{_uid}", [TC, 128, DI], F16).ap()
        ca_dram = nc.dram_tensor(f"ca_bcast_{_uid}", [TC, H * CH], F32).ap()
        daend_dram = nc.dram_tensor(f"daend_bcast_{_uid}", [TC, H], F16).ap()

        with tc.tile_pool(name="hTp", bufs=1) as hTp, \
             tc.tile_pool(name="wzp", bufs=1) as wzp:
            # hT layout: [d_lo, tc, kd, t_lo] — per-chunk transpose writes the
            # contiguous [128, 8, 128] block hT[:, tc]
            hT = hTp.tile([128, TC, 8, 128], F16)  # 2 MB
            wz = wzp.tile([128, 4, 8, 512], F16)

            # ---------- phase 0: PE warmup chain (keep HAM at K=8/8) ----------
            with tc.tile_pool(name="warm", bufs=1, space="PSUM") as warmp:
                wps = warmp.tile([128, 512], F32, tag="wps")
                for i in range(40):
                    nc.tensor.matmul(wps[:], lhsT=ident16[:], rhs=warm_rhs[:],
                                     start=(i == 0), stop=(i == 39))

            # ---------- phase 1: LN(u) -> h (f16), PE-transpose -> hT ----------
            with tc.tile_pool(name="ph1u", bufs=8) as p1u, \
                 tc.tile_pool(name="ph1", bufs=3) as p1, \
                 tc.tile_pool(name="ph1s", bufs=3) as p1s, \
                 tc.tile_pool(name="ph1ps", bufs=2, space="PSUM") as p1ps:
                u_ts = []
                for t_c in range(TC):
                    u_t = p1u.tile([128, DM], F32, tag="u", name=f"u{t_c}")
                    nc.gpsimd.dma_start(u_t[:], u_d[t_c * 128:(t_c + 1) * 128, :])
                    u_ts.append(u_t)
                # prefetch z weights (4 MB, needed ~100us later) after u data
                for eq in range(4):
                    nc.gpsimd.dma_start(wz[:, eq], w_in_d[:, eq * 512:(eq + 1) * 512]
                                        .rearrange("(kd p) e -> p kd e", p=128))
                for t_c in range(TC):
                    u_t = u_ts[t_c]
                    ssum = p1s.tile([128, 1], F32, tag="ssum")
                    nc.vector.tensor_reduce(ssum[:], u_t[:], axis=AX.X, op=ALU.add)
                    sq = p1.tile([128, DM], F32, tag="sq")
                    ssq = p1s.tile([128, 1], F32, tag="ssq")
                    nc.scalar.activation(sq[:], u_t[:], AF.Square, accum_out=ssq[:])
                    nmean = p1s.tile([128, 1], F32, tag="nmean")
                    nc.vector.tensor_scalar_mul(nmean[:], ssum[:], -1.0 / DM)
                    var = p1s.tile([128, 1], F32, tag="var")
                    nc.vector.tensor_tensor(var[:], nmean[:], nmean[:], op=ALU.mult)
                    nc.vector.scalar_tensor_tensor(var[:], ssq[:], 1.0 / DM, var[:],
                                                   op0=ALU.mult, op1=ALU.subtract)
                    nc.vector.tensor_scalar_add(var[:], var[:], EPS)
                    rstd = p1s.tile([128, 1], F32, tag="rstd")
                    _fast_rsqrt(nc, p1s, rstd[:], var[:], magic_t[:], (128, 1), "ln")
                    bias2 = p1s.tile([128, 1], F32, tag="bias2")
                    nc.vector.tensor_tensor(bias2[:], nmean[:], rstd[:], op=ALU.mult)
                    h_t = p1.tile([128, DM], F16, tag="h")
                    nc.vector.tensor_scalar(h_t[:], u_t[:], rstd[:], bias2[:],
                                            op0=ALU.mult, op1=ALU.add)
                    tp = p1ps.tile([128, 8, 128], F16, tag="tr1")
                    for kd in range(8):
                        nc.tensor.transpose(tp[:, kd, :],
                                            h_t[:, kd * 128:(kd + 1) * 128], ident16[:])
                    nc.scalar.activation(hT[:, t_c].rearrange("p a b -> p (a b)"),
                                         tp[:].rearrange("p a b -> p (a b)"), AF.Copy)

            # ---------- phase 2: in_proj xBC/dt + DVE conv + DMA transposes ----
            with tc.tile_pool(name="p2w", bufs=6) as wp, \
                 tc.tile_pool(name="p2", bufs=3) as p2, \
                 tc.tile_pool(name="p2ps", bufs=2, space="PSUM") as pps, \
                 tc.tile_pool(name="p2pt", bufs=2, space="PSUM") as ppt:
                for ec in range(19):
                    e0 = DI + ec * 128
                    m = 128 if ec < 18 else 32
                    ps = pps.tile([128, 2, 512], F32, tag="px")
                    wt = wp.tile([128, 8, 128], F16, tag="w")
                    nc.scalar.dma_start(wt[:], w_in_d[:, e0:e0 + 128]
                                        .rearrange("(kd p) e -> p kd e", p=128))
                    for th in range(2):
                        for kd in range(8):
                            nc.tensor.matmul(ps[:m, th], lhsT=wt[:, kd, :m],
                                             rhs=hT[:, th * 4:(th + 1) * 4, kd, :],
                                             start=(kd == 0), stop=(kd == 7))
                    if ec == 18:
                        # softplus(x + dt_bias) = ln(1 + exp(x + dt_bias))
                        nc.scalar.activation(dt_sb[:], ps[:32].rearrange("p a b -> p (a b)"),
                                             AF.Exp, bias=dt_bias[:])
                        nc.scalar.activation(dt_sb[:], dt_sb[:], AF.Ln, bias=1.0)
                        continue
                    xr = p2.tile([128, 3 + L], F16, tag="xraw")
                    nc.gpsimd.memset(xr[:, 0:3], 0.0)
                    nc.scalar.activation(xr[:, 3:3 + L], ps[:].rearrange("p a b -> p (a b)"),
                                         AF.Copy)
                    # depthwise causal conv: 4 shifted MACs on DVE
                    xc = p2.tile([128, L], F16, tag="xconv")
                    nc.vector.tensor_scalar_mul(xc[:], xr[:, 0:L], conv_wt[:, ec, 0:1])
                    for k in range(1, 4):
                        nc.vector.scalar_tensor_tensor(xc[:], xr[:, k:k + L],
                                                       conv_wt[:, ec, k:k + 1], xc[:],
                                                       op0=ALU.mult, op1=ALU.add)
                    def _silu_conv(dst):
                        if USE_SILU:
                            nc.scalar.activation(dst, xc[:], AF.Silu,
                                                 bias=conv_bt[:, ec:ec + 1])
                        else:
                            sg = p2.tile([128, L], F16, tag="sg")
                            nc.scalar.activation(sg[:], xc[:], AF.Sigmoid,
                                                 bias=conv_bt[:, ec:ec + 1])
                            nc.vector.scalar_tensor_tensor(dst, xc[:],
                                                           conv_bt[:, ec:ec + 1],
                                                           sg[:], op0=ALU.add, op1=ALU.mult)
                    if ec <= 15:
                        xa = p2.tile([128, L], F16, tag="xact")
                        _silu_conv(xa[:])
                        tp2 = ppt.tile([128, 8, 128], F16, tag="tr2")
                        for tcb in range(8):
                            nc.tensor.transpose(tp2[:, tcb, :],
                                                xa[:, tcb * 128:(tcb + 1) * 128], ident16[:])
                        nc.scalar.activation(X_t[:, ec].rearrange("p a b -> p (a b)"),
                                             tp2[:].rearrange("p a b -> p (a b)"), AF.Copy)
                    elif ec == 16:
                        _silu_conv(BT_sb[:])
                        tp2 = ppt.tile([128, 8, 128], F16, tag="tr2")
                        for tcb in range(8):
                            nc.tensor.transpose(tp2[:, tcb, :],
                                                BT_sb[:, tcb * 128:(tcb + 1) * 128], ident16[:])
                        nc.scalar.activation(B_t[:].rearrange("p a b -> p (a b)"),
                                             tp2[:].rearrange("p a b -> p (a b)"), AF.Copy)
                    else:
                        _silu_conv(CT_sb[:])

            # ---------- phase 3: z GEMM -> silu_z -> spill ----------
            with tc.tile_pool(name="p3b", bufs=4) as p3b, \
                 tc.tile_pool(name="p3ps", bufs=2, space="PSUM") as pz:
                # prefetch out_proj weights during phase 3
                nc.gpsimd.dma_start(w_out_sb[:], w_out_d.rearrange("(eo p) d -> p eo d", p=128))
                for eq in range(4):
                    for tg in range(4):
                        pss = pz.tile([128, 2, 512], F32, tag="pz")
                        for j in range(2):
                            t_c = tg * 2 + j
                            for kd in range(8):
                                nc.tensor.matmul(pss[:, j],
                                                 lhsT=hT[:, t_c, kd, :],
                                                 rhs=wz[:, eq, kd],
                                                 start=(kd == 0), stop=(kd == 7))
                        for j in range(2):
                            t_c = tg * 2 + j
                            zb = p3b.tile([128, 512], F16, tag="zb")
                            if USE_SILU:
                                nc.scalar.activation(zb[:], pss[:, j], AF.Silu)
                            else:
                                zsg = p3b.tile([128, 512], F16, tag="zsg")
                                nc.scalar.activation(zsg[:], pss[:, j], AF.Sigmoid)
                                nc.vector.tensor_tensor(zb[:], pss[:, j], zsg[:], op=ALU.mult)
                            nc.gpsimd.dma_start(sz_dram[t_c, :, eq * 512:(eq + 1) * 512], zb[:])
        # hT + wz pools closed here

        # ---------- dt/cA machinery ----------
        with tc.tile_pool(name="pdt", bufs=2) as pdt, \
             tc.tile_pool(name="pdtps", bufs=4, space="PSUM") as pdtps:
            dtA = pdt.tile([32, L], F32, tag="dtA")
            nc.vector.tensor_scalar_mul(dtA[:], dt_sb[:], a_neg[:])
            for t_c in range(TC):
                sl = slice(t_c * 128, (t_c + 1) * 128)
                nc.vector.tensor_tensor_scan(cA_row[:, t_c, :], dtA[:, sl], dtA[:, sl],
                                             initial=0.0, op0=ALU.add, op1=ALU.bypass)
                pdts = pdtps.tile([128, 32], F32, tag="pq")
                nc.tensor.transpose(pdts[:], dt_sb[:, sl], ident32[:32, :32])
                nc.any.tensor_copy(out=dt_T[:, t_c, :], in_=pdts[:])
                wr = pdt.tile([32, CH], F32, tag="wr")
                nc.scalar.activation(wr[:], cA_row[:, t_c, :], AF.Exp, scale=-1.0,
                                     bias=cA_row[:, t_c, 127:128])
                pdts2 = pdtps.tile([128, 32], F32, tag="pq")
                nc.tensor.transpose(pdts2[:], wr[:], ident32[:32, :32])
                nc.any.tensor_copy(out=w_T[:, t_c, :], in_=pdts2[:])
                pdts3 = pdtps.tile([128, 32], F32, tag="pq")
                nc.tensor.transpose(pdts3[:], cA_row[:, t_c, :], ident32[:32, :32])
                nc.vector.tensor_scalar_mul(cA_colneg[:, t_c, :], pdts3[:], -1.0)
                nc.sync.dma_start(ca_dram[t_c], cA_row[:, t_c, :])
                daend_2d = pdt.tile([32, 1], F16, tag="daend_2d")
                nc.scalar.activation(daend_2d[:], cA_row[:, t_c, 127:128], AF.Exp)
                nc.sync.dma_start(daend_dram[t_c], daend_2d[:])
            # ln(dt) in [tp, tc, h]; colmix = -cA[s] + ln dt[s]; E0 = exp(cA[t])
            lndt = pdt.tile([128, TC, H], F32, tag="lndt")
            nc.scalar.activation(lndt[:].rearrange("p a b -> p (a b)"),
                                 dt_T[:].rearrange("p a b -> p (a b)"), AF.Ln)
            nc.vector.tensor_tensor(colmix[:], lndt[:], cA_colneg[:], op=ALU.add)
            nc.scalar.activation(E0col[:].rearrange("p a b -> p (a b)"),
                                 cA_colneg[:].rearrange("p a b -> p (a b)"),
                                 AF.Exp, scale=-1.0)

        # ---------- phase 4: scan + gating + rmsnorm + out_proj ----------
        with tc.tile_pool(name="p4ca", bufs=2) as p4ca, \
             tc.tile_pool(name="p4rep", bufs=2) as p4rep, \
             tc.tile_pool(name="p4sz", bufs=2) as p4sz, \
             tc.tile_pool(name="p4big", bufs=2) as p4big, \
             tc.tile_pool(name="p4mt", bufs=2) as p4mt, \
             tc.tile_pool(name="p4s", bufs=2) as p4s, \
             tc.tile_pool(name="p4x", bufs=2) as p4x, \
             tc.tile_pool(name="p4g", bufs=1) as p4g, \
             tc.tile_pool(name="p4y", bufs=1) as p4y, \
             tc.tile_pool(name="p4o", bufs=2) as p4o, \
             tc.tile_pool(name="psgt", bufs=1, space="PSUM") as psgt, \
             tc.tile_pool(name="psy", bufs=1, space="PSUM") as psy, \
             tc.tile_pool(name="psst", bufs=1, space="PSUM") as psst, \
             tc.tile_pool(name="pstr", bufs=1, space="PSUM") as pstr, \
             tc.tile_pool(name="pso", bufs=1, space="PSUM") as pso:
            def fetch(i):
                szb = p4sz.tile([128, DI], F16, tag="szsb")
                nc.scalar.dma_start(szb[:], sz_dram[i])
                dae = p4rep.tile([128, H], F16, tag="daendrep")
                nc.sync.dma_start(dae[:],
                                  daend_dram[i:i + 1, :].partition_broadcast(128)[:, 0, :])
                car = p4ca.tile([128, H, CH], F32, tag="carep")
                nc.sync.dma_start(car[:].rearrange("p a b -> p (a b)"),
                                  ca_dram[i:i + 1, :].partition_broadcast(128)[:, 0, :])
                return car, dae, szb

            nxt = fetch(0)
            S_prev = None
            for t_c in range(TC):
                tsl = slice(t_c * 128, (t_c + 1) * 128)
                ca_rep, daend_rep, sz_sb = nxt
                if t_c + 1 < TC:
                    nxt = fetch(t_c + 1)
                # x * dt * decay-to-end
                dtw = p4x.tile([128, H], F16, tag="dtw")
                nc.vector.tensor_tensor(dtw[:], dt_T[:, t_c, :], w_T[:, t_c, :],
                                        op=ALU.mult)
                xch = p4x.tile([128, H, PH], F16, tag="xchk")
                nc.vector.tensor_tensor(
                    xch[:].rearrange("p (a b) q -> p a b q", a=16),
                    X_t[:, :, t_c, :].rearrange("p a (b q) -> p a b q", q=PH),
                    dtw[:].rearrange("p (a b) -> p a b", a=16)[:, :, :, None]
                    .to_broadcast((128, 16, 2, PH)),
                    op=ALU.mult)
                S_new = p4s.tile([128, H, PH], F16, tag="S")
                if t_c > 0:
                    nc.vector.tensor_tensor(S_new[:], S_prev[:],
                                            daend_rep[:, :, None].to_broadcast((128, H, PH)),
                                            op=ALU.mult)
                # add colmix -> f16 (DVE), then causal mask (Pool)
                ca16 = p4big.tile([128, H, CH], F16, tag="ca16")
                nc.vector.tensor_tensor(
                    ca16[:], ca_rep[:],
                    colmix[:, t_c, :, None].to_broadcast((128, H, CH)),
                    op=ALU.add)
                nc.gpsimd.affine_select(out=ca16[:], in_=ca16[:],
                                        pattern=[[0, H], [1, CH]],
                                        compare_op=ALU.is_ge, fill=-1000.0, base=0,
                                        channel_multiplier=-1)
                nc.scalar.activation(ca16[:].rearrange("p a b -> p (a b)"),
                                     ca16[:].rearrange("p a b -> p (a b)"), AF.Exp)
                # Gt (shared across heads)
                gt_ps = psgt.tile([128, CH], F32, tag="gt")
                nc.tensor.matmul(gt_ps[:], lhsT=BT_sb[:, tsl], rhs=CT_sb[:, tsl],
                                 start=True, stop=True)
                gt_sb = p4x.tile([128, CH], F16, tag="gts")
                nc.scalar.activation(gt_sb[:], gt_ps[:], AF.Copy)
                # M = E' * gt  (D handled in gating)
                Mt = p4mt.tile([128, H, CH], F16, tag="Mt")
                nc.vector.tensor_tensor(Mt[:], ca16[:],
                                        gt_sb[:, None, :].to_broadcast((128, H, CH)),
                                        op=ALU.mult)
                g16 = p4g.tile([128, DI], F16, tag="g16")
                for q in range(2):
                    hsl = slice(q * 16, (q + 1) * 16)
                    esl = slice(q * 1024, (q + 1) * 1024)
                    # state GEMM for this half (1 PSUM bank, 2 sequential halves)
                    for j in range(2):
                        st = psst.tile([128, 512], F32, tag="st")
                        nc.tensor.matmul(st[:], lhsT=B_t[:, t_c, :],
                                         rhs=xch[:].rearrange("p a b -> p (a b)")[:, q * 1024 + j * 512: q * 1024 + (j + 1) * 512],
                                         start=True, stop=True)
                        hs8 = slice(q * 16 + j * 8, q * 16 + (j + 1) * 8)
                        if t_c == 0:
                            nc.vector.tensor_copy(out=S_new[:, hs8, :],
                                                  in_=st[:].rearrange("p (h q) -> p h q", q=PH))
                        else:
                            nc.vector.tensor_tensor(S_new[:, hs8, :], S_new[:, hs8, :],
                                                    st[:].rearrange("p (h q) -> p h q", q=PH),
                                                    op=ALU.add)
                    # intra-chunk: y1 = M^T x per head
                    y1_ps = psy.tile([128, 16, PH], F32, tag="y1")
                    for hh in range(16):
                        h = q * 16 + hh
                        nc.tensor.matmul(y1_ps[:, hh, :], lhsT=Mt[:, h, :],
                                         rhs=X_t[:, h // 2, t_c, (h % 2) * PH:(h % 2 + 1) * PH],
                                         start=True, stop=True)
                    xD = p4y.tile([128, 16, PH], F16, tag="xD")
                    nc.vector.tensor_tensor(
                        xD[:].rearrange("p a b -> p (a b)").rearrange("p (a b) -> p a b", a=8),
                        X_t[:, q * 8:(q + 1) * 8, t_c, :],
                        D_rep[:, esl].rearrange("p (a b) -> p a b", a=8), op=ALU.mult)
                    if t_c > 0:
                        # inter-chunk: y2 = C^T S_prev, batched over heads
                        y2_ps = psy.tile([128, 2, 512], F32, tag="y2")
                        for j in range(2):
                            nc.tensor.matmul(y2_ps[:, j, :], lhsT=CT_sb[:, tsl],
                                             rhs=S_prev[:].rearrange("p a b -> p (a b)")[:, q * 1024 + j * 512: q * 1024 + (j + 1) * 512],
                                             start=True, stop=True)
                        y2c = p4y.tile([128, 16, PH], F16, tag="y2c")
                        nc.scalar.activation(y2c[:].rearrange("p a b -> p (a b)"),
                                             y2_ps[:].rearrange("p a b -> p (a b)"),
                                             AF.Copy)
                        t1 = p4y.tile([128, 16, PH], F16, tag="t1")
                        nc.vector.tensor_tensor(
                            t1[:], y2c[:],
                            E0col[:, t_c, hsl, None].to_broadcast((128, 16, PH)),
                            op=ALU.mult)
                        nc.vector.tensor_tensor(t1[:], t1[:],
                                                y1_ps[:].rearrange("p a (h q) -> p (a h) q", q=PH),
                                                op=ALU.add)
                        nc.vector.tensor_tensor(t1[:], t1[:], xD[:], op=ALU.add)
                        nc.vector.tensor_tensor(g16[:, esl],
                                                t1[:].rearrange("p a b -> p (a b)"),
                                                sz_sb[:, esl], op=ALU.mult)
                    else:
                        t1 = p4y.tile([128, 16, PH], F16, tag="t1")
                        nc.vector.tensor_tensor(t1[:],
                                                y1_ps[:].rearrange("p a (h q) -> p (a h) q", q=PH),
                                                xD[:], op=ALU.add)
                        nc.vector.tensor_tensor(g16[:, esl],
                                                t1[:].rearrange("p a b -> p (a b)"),
                                                sz_sb[:, esl], op=ALU.mult)
                S_prev = S_new
                # --- rmsnorm stats (rstd applied at out_proj output) ---
                gsq = p4g.tile([128, 1024], F32, tag="gsq")
                sq1 = p4g.tile([128, 1], F32, tag="sq1")
                sq2 = p4g.tile([128, 1], F32, tag="sq2")
                nc.scalar.activation(gsq[:], g16[:, 0:1024], AF.Square, accum_out=sq1[:])
                nc.scalar.activation(gsq[:], g16[:, 1024:2048], AF.Square, accum_out=sq2[:])
                nc.vector.tensor_tensor(sq1[:], sq1[:], sq2[:], op=ALU.add)
                msq = p4g.tile([128, 1], F32, tag="msq")
                nc.vector.tensor_scalar(msq[:], sq1[:], 1.0 / DI, EPS,
                                        op0=ALU.mult, op1=ALU.add)
                rstd = p4g.tile([128, 1], F32, tag="rstd")
                _fast_rsqrt(nc, p4g, rstd[:], msq[:], magic_t[:], (128, 1), "rms")
                # --- transpose g16 -> [e, t] on PE ---
                gT = p4o.tile([128, 16, CH], F16, tag="gT")
                for eg in range(2):
                    tp4 = pstr.tile([128, 8, 128], F16, tag="tr4")
                    for j in range(8):
                        eo = eg * 8 + j
                        nc.tensor.transpose(tp4[:, j, :],
                                            g16[:, eo * 128:(eo + 1) * 128], ident16[:])
                    nc.scalar.activation(gT[:, eg * 8:(eg + 1) * 8, :].rearrange("p a b -> p (a b)"),
                                         tp4[:].rearrange("p a b -> p (a b)"), AF.Copy)
                # --- out_proj (+ deferred rstd scale) ---
                for dh in range(2):
                    po = pso.tile([128, 512], F32, tag="po")
                    for eo in range(16):
                        nc.tensor.matmul(po[:], lhsT=gT[:, eo, :],
                                         rhs=w_out_sb[:, eo, dh * 512:(dh + 1) * 512],
                                         start=(eo == 0), stop=(eo == 15))
                    ob = p4o.tile([128, 512], F16, tag="ob")
                    nc.scalar.activation(ob[:], po[:], AF.Copy, scale=rstd[:])
                    nc.gpsimd.dma_start(out_d[tsl, dh * 512:(dh + 1) * 512], ob[:])


_NC_CACHE = {}

N_CORES = 8
BSZ = 4


def _get_nc():
    if "nc" not in _NC_CACHE:
        nc = bacc.Bacc("TRN2", target_bir_lowering=False, debug=False,
                       num_devices=N_CORES)
        _NC_CACHE["nc"] = _build(nc)
    return _NC_CACHE["nc"]


def _get_runner():
    """Build the jitted SPMD callable once so repeat kernel() calls skip
    retrace + NEFF recompile (run_bass_via_pjrt builds a fresh closure per
    call, defeating the jit cache)."""
    if "runner" not in _NC_CACHE:
        _NC_CACHE["runner"] = _make_runner(_get_nc())
    return _NC_CACHE["runner"]


def _make_runner(nc):
    import jax
    from jax.sharding import Mesh, PartitionSpec
    from jax.experimental.shard_map import shard_map
    from concourse import bass2jax, mybir as _mb

    bass2jax.install_neuronx_cc_hook()
    partition_name = nc.partition_id_tensor.name if nc.partition_id_tensor else None
    in_names, out_names, out_avals, zero_outs = [], [], [], []
    for alloc in nc.m.functions[0].allocations:
        if not isinstance(alloc, _mb.MemoryLocationSet):
            continue
        name = alloc.memorylocations[0].name
        if alloc.kind == "ExternalInput":
            if name != partition_name:
                in_names.append(name)
        elif alloc.kind == "ExternalOutput":
            shape = tuple(alloc.tensor_shape)
            dtype = _mb.dt.np(alloc.dtype)
            out_names.append(name)
            out_avals.append(jax.core.ShapedArray(shape, dtype))
            zero_outs.append(np.zeros(shape, dtype))
    n_params = len(in_names)
    n_outs = len(out_avals)
    all_in_names = list(in_names) + list(out_names)
    if partition_name is not None:
        all_in_names.append(partition_name)
    donate = tuple(range(n_params, n_params + n_outs))

    def _bodyfn(*args):
        operands = list(args)
        if partition_name is not None:
            operands.append(bass2jax.partition_id_tensor())
        outs = bass2jax._bass_exec_p.bind(
            *operands,
            out_avals=tuple(out_avals),
            in_names=tuple(all_in_names),
            out_names=tuple(out_names),
            lowering_input_output_aliases=(),
            sim_require_finite=True,
            sim_require_nnan=True,
            nc=nc,
        )
        return tuple(outs)

    devices = jax.devices()[:N_CORES]
    mesh = Mesh(np.asarray(devices), ("core",))
    in_specs = (PartitionSpec("core"),) * (n_params + n_outs)
    out_specs = (PartitionSpec("core"),) * n_outs
    sharded = jax.jit(
        shard_map(_bodyfn, mesh=mesh, in_specs=in_specs, out_specs=out_specs,
                  check_rep=False),
        donate_argnums=donate, keep_unused=True)

    def run(in_maps):
        per_core = [[np.asarray(m[name]) for name in in_names] for m in in_maps]
        concat_in = [np.concatenate([per_core[c][i] for c in range(N_CORES)], axis=0)
                     for i in range(n_params)]
        concat_zeros = [np.zeros((N_CORES * z.shape[0], *z.shape[1:]), z.dtype)
                        for z in zero_outs]
        out_arrs = sharded(*concat_in, *concat_zeros)
        return [{name: np.asarray(out_arrs[i]).reshape(N_CORES, *out_avals[i].shape)[c]
                 for i, name in enumerate(out_names)}
                for c in range(N_CORES)]

    def make_device_exec(in_maps):
        """For timing: stage inputs on-device once; returns f() that runs one
        execution with on-device zero outputs and blocks until done."""
        from jax.sharding import NamedSharding
        per_core = [[np.asarray(m[name]) for name in in_names] for m in in_maps]
        concat_in = [np.concatenate([per_core[c][i] for c in range(N_CORES)], axis=0)
                     for i in range(n_params)]
        shard = NamedSharding(mesh, PartitionSpec("core"))
        dev_in = [jax.device_put(a, shard) for a in concat_in]
        zero_shapes = [(N_CORES * z.shape[0], *z.shape[1:]) for z in zero_outs]
        zdtypes = [z.dtype for z in zero_outs]
        import jax.numpy as jnp
        mk_zeros = jax.jit(
            lambda: tuple(jnp.zeros(s, d) for s, d in zip(zero_shapes, zdtypes)),
            out_shardings=tuple(shard for _ in zero_shapes))

        def exec_once():
            zs = mk_zeros()
            jax.block_until_ready(zs)
            import time as _t
            t0 = _t.perf_counter()
            outs = sharded(*dev_in, *zs)
            jax.block_until_ready(outs)
            return _t.perf_counter() - t0
        return exec_once

    run.make_device_exec = make_device_exec
    return run


def _smart_flip(X, lengths):
    B, Ln, _ = X.shape
    r = np.arange(Ln)[None, :]
    pos = np.where(r < lengths[:, None], lengths[:, None] - 1 - r, r)
    return np.take_along_axis(X, pos[:, :, None], axis=1)


def _dir_params(in_proj_w, out_proj_w, conv_w, conv_b, dt_bias, A_log, D, norm_w):
    w_in = np.zeros((DM, EPAD), np.float16)
    w_in[:, :EIN] = in_proj_w.T.astype(np.float16)
    # out_proj with norm_w folded in: w_out[e, d] = norm_w[e] * out_proj_w[d, e]
    w_out = np.ascontiguousarray(out_proj_w.T).astype(np.float64)
    w_out = (w_out * norm_w.astype(np.float64)[:, None]).astype(np.float16)
    d_rep = np.broadcast_to(
        np.repeat(D.astype(np.float16), PH)[None, :], (128, DI)).copy()
    return {
        "w_in": w_in,
        "w_out": w_out,
        "conv_wt": np.ascontiguousarray(
            conv_w.reshape(18, 128, 4).transpose(1, 0, 2)).astype(np.float32),
        "conv_bt": np.ascontiguousarray(conv_b.reshape(18, 128).T.astype(np.float32)),
        "dt_bias": dt_bias.reshape(32, 1).astype(np.float32),
        "a_neg": (-np.exp(A_log.astype(np.float64))).astype(np.float32).reshape(32, 1),
        "d_rep": d_rep,
    }


def kernel(hidden_states, src_key_padding_mask, in_proj_w, out_proj_w,
           conv_w_f, conv_b_f, dt_bias_f, A_log_f, D_f, norm_w_f,
           conv_w_r, conv_b_r, dt_bias_r, A_log_r, D_r, norm_w_r):
    hidden_states = np.asarray(hidden_states, np.float32)
    mask = np.asarray(src_key_padding_mask)
    lengths = (~mask).sum(axis=1)
    rev = _smart_flip(hidden_states, lengths)

    pf = _dir_params(np.asarray(in_proj_w), np.asarray(out_proj_w),
                     np.asarray(conv_w_f), np.asarray(conv_b_f),
                     np.asarray(dt_bias_f), np.asarray(A_log_f),
                     np.asarray(D_f), np.asarray(norm_w_f))
    pr = _dir_params(np.asarray(in_proj_w), np.asarray(out_proj_w),
                     np.asarray(conv_w_r), np.asarray(conv_b_r),
                     np.asarray(dt_bias_r), np.asarray(A_log_r),
                     np.asarray(D_r), np.asarray(norm_w_r))

    run = _get_runner()
    in_maps = []
    for core in range(N_CORES):
        d, b = divmod(core, BSZ)
        u = hidden_states[b] if d == 0 else rev[b]
        m = dict(pf if d == 0 else pr)
        m["u"] = np.ascontiguousarray(u)
        in_maps.append(m)
    results = run(in_maps)
    out_f = np.stack([results[b]["out"] for b in range(BSZ)])
    out_r = np.stack([results[BSZ + b]["out"] for b in range(BSZ)])
    out_r = _smart_flip(out_r, lengths)
    out = (out_f.astype(np.float64) + out_r.astype(np.float64)) / 2.0
    mu = out.mean(-1, keepdims=True)
    v = out.var(-1, keepdims=True)
    out = (out - mu) / np.sqrt(v + EPS)
    return out.astype(np.float32)
